# revision 10
# baseline (speedup 1.0000x reference)
"""Trainium2 Bass kernel: multi-head attention with RoPE + gated prompt
injection (nn_Attention_28080496181816), sharded over 8 NeuronCores.

Sharding: tensor-parallel over heads. Core c owns heads [4c, 4c+4):
  - wq/wk/wv column-sharded (per-head), o-proj via AllGather of the
    per-core attention outputs + column-sharded wo matmul.
  - Host-side unshard is a pure concatenation of output column slices.

Layout: "T-major" — activations live as [feature, token] on device so
every matmul contraction lands on the partition axis with no on-device
transposes. RoPE pairs are made contiguous by permuting wq/wk rows
(per head: even hd dims then odd hd dims) on the host.

Attention runs in 512-wide query groups with variable-width score
matmuls derived from the runtime mask structure (causal -> exact
lower-triangle work). The emission order software-pipelines phases so
the in-order PE stream interleaves attention with the next batch's
projections and the previous batch's output projection.
"""

import math
import os
import sys
import types

import numpy as np
import ml_dtypes

# --- optional NTFF profile hook shim (only needed if BASS_TRACE is set;
# the stock image lacks antenv.axon_hooks) ---
try:
    import antenv.axon_hooks  # noqa: F401
except Exception:
    try:
        import antenv
        _m = types.ModuleType("antenv.axon_hooks")
        _hook = [None]
        _m.set_axon_ntff_profile_hook = lambda h: _hook.__setitem__(0, h)
        _m.get_axon_ntff_profile_hook = lambda: _hook[0]
        sys.modules["antenv.axon_hooks"] = _m
        antenv.axon_hooks = _m
        from trn_agent_boot.trn_boot import _ntff_profile_via_ctypes
        _p = _ntff_profile_via_ctypes("/opt/axon/libaxon_pjrt.so")
        if _p is not None:
            _m.set_axon_ntff_profile_hook(_p)
    except Exception:
        pass

import concourse.bacc as bacc
import concourse.mybir as mybir
import concourse.tile as tile
from concourse import bass_utils

BF16 = mybir.dt.bfloat16
F32 = mybir.dt.float32
NPBF16 = ml_dtypes.bfloat16

B, S, D, H, HD, PL = 2, 1024, 4096, 32, 128, 10
NC = 8              # cores
HLOC = H // NC      # 4 heads per core
DLOC = HLOC * HD    # 512
T = B * S           # 2048
NDX = D // 128      # 32 contraction blocks
NQT = S // 128      # 8 query tiles per batch
NQG = NQT // 4      # 2 query groups of 512
SCALE = 1.0 / math.sqrt(HD)

_PROG_CACHE = {}


def _analyze_mask(mask):
    """Classify each 128x128 tile of the additive mask: skip (fully
    masked), clear (all zero) or mixed (ship the transposed, pre-scaled
    tile). Deduplicates mixed tiles."""
    mq = np.asarray(mask).reshape(S, S)
    plan = []
    uniq = {}
    mlist = []
    for qi in range(NQT):
        row = []
        for kb in range(NQT):
            sub = mq[qi * 128:(qi + 1) * 128, kb * 128:(kb + 1) * 128]
            if np.all(sub <= -1e8):
                continue
            if np.all(sub == 0):
                row.append((kb, None))
                continue
            tt = np.ascontiguousarray(sub.T.astype(np.float32) / SCALE)
            key = tt.tobytes()
            if key not in uniq:
                uniq[key] = len(mlist)
                mlist.append(tt)
            row.append((kb, uniq[key]))
        plan.append(row)
    return plan, mlist


def _group_plan(plan, n_mtiles):
    """512-wide query groups. Per group: list of (kb, q0, q1, adds) with
    q0..q1 the covered query quarters and adds = [(quarter, mtile_idx)];
    mtile_idx == n_mtiles selects the -inf tile. The first kb of each
    group always spans the full group so PSUM has_written is set."""
    NEG = n_mtiles
    plan2 = []
    for qg in range(NQG):
        qmode = []
        for q in range(4):
            qmode.append(dict(plan[qg * 4 + q]))
        live = sorted(set().union(*[set(d.keys()) for d in qmode]))
        entries = []
        for j, kb in enumerate(live):
            pres = [kb in qmode[q] for q in range(4)]
            if j == 0:
                q0, q1 = 0, 3
            else:
                q0 = min(q for q in range(4) if pres[q])
                q1 = max(q for q in range(4) if pres[q])
            adds = []
            for q in range(q0, q1 + 1):
                if not pres[q]:
                    adds.append((q, NEG))
                elif qmode[q][kb] is not None:
                    adds.append((q, qmode[q][kb]))
            entries.append((kb, q0, q1, tuple(adds)))
        plan2.append(tuple(entries))
    return plan2


def _build_program(plan2, n_mt):
    """Build + compile the SPMD program (identical on all 8 cores).
    n_mt counts mask tiles INCLUDING the trailing -inf tile."""
    nc = bacc.Bacc("TRN2", target_bir_lowering=False, debug=False, num_devices=NC)

    # p-major host layouts so each logical group is ONE big DMA
    xt = nc.dram_tensor("xt", [4, 128, NDX, 512], BF16, kind="ExternalInput")
    wqt = nc.dram_tensor("wqt", [HLOC, 128, NDX, 128], BF16, kind="ExternalInput")
    wkt = nc.dram_tensor("wkt", [HLOC, 128, NDX, 128], BF16, kind="ExternalInput")
    wvt = nc.dram_tensor("wvt", [128, NDX, DLOC], BF16, kind="ExternalInput")
    wot = nc.dram_tensor("wot", [128, NDX, DLOC], BF16, kind="ExternalInput")
    pt = nc.dram_tensor("pt", [128, NDX, PL], BF16, kind="ExternalInput")
    cosT = nc.dram_tensor("cosT", [64, S], F32, kind="ExternalInput")
    sinT = nc.dram_tensor("sinT", [64, S], F32, kind="ExternalInput")
    gates = nc.dram_tensor("gates", [1, HLOC], F32, kind="ExternalInput")
    mtiles = nc.dram_tensor("mtiles", [n_mt, 128, 128], F32, kind="ExternalInput")
    out_d = nc.dram_tensor("out", [T, DLOC], F32, kind="ExternalOutput")

    AF = mybir.ActivationFunctionType
    OP = mybir.AluOpType

    with tile.TileContext(nc) as tc:
        with (
            tc.tile_pool(name="const", bufs=1) as cpool,
            tc.tile_pool(name="wres", bufs=1) as wres,
            tc.tile_pool(name="stream", bufs=1) as sp,
            tc.tile_pool(name="act", bufs=1) as ap,
            tc.tile_pool(name="psum", bufs=1, space="PSUM") as pp,
            tc.tile_pool(name="dram", bufs=1, space="DRAM") as dp,
        ):
            # ---- persistent constants / weights ----
            cos_sb = cpool.tile([64, S], F32, tag="cos")
            sin_sb = cpool.tile([64, S], F32, tag="sin")
            nc.sync.dma_start(cos_sb[:], cosT[:])
            nc.sync.dma_start(sin_sb[:], sinT[:])
            gates_sb = cpool.tile([1, HLOC], F32, tag="gates")
            nc.sync.dma_start(gates_sb[:], gates[:])
            mt_sb = []
            for i in range(n_mt):
                t = cpool.tile([128, 128], F32, tag=f"mt{i}", name=f"mt{i}")
                nc.sync.dma_start(t[:], mtiles[i])
                mt_sb.append(t)
            ones_col = cpool.tile([128, 1], BF16, tag="ones_col")
            nc.vector.memset(ones_col[:], 1.0)

            # wv / wo resident: 4 tiles each of [128, 8*512]
            # (DMAs issued on the GpSimd queue after the first projection
            # group so they don't delay the critical-path x/wq loads)
            wv_sb = [wres.tile([128, 8 * DLOC], BF16, tag=f"wv{j}",
                               name=f"wv{j}") for j in range(4)]
            wo_sb = [wres.tile([128, 8 * DLOC], BF16, tag=f"wo{j}",
                               name=f"wo{j}") for j in range(4)]

            def emit_wvwo_loads():
                for j in range(4):
                    nc.gpsimd.dma_start(wv_sb[j][:], wvt[:, 8 * j:8 * (j + 1), :])
                for j in range(4):
                    nc.gpsimd.dma_start(wo_sb[j][:], wot[:, 8 * j:8 * (j + 1), :])

            def wv_sl(i):
                return wv_sb[i // 8][:, (i % 8) * DLOC:(i % 8 + 1) * DLOC]

            def wo_sl(i):
                return wo_sb[i // 8][:, (i % 8) * DLOC:(i % 8 + 1) * DLOC]

            pt_sb = cpool.tile([128, NDX * PL], BF16, tag="pt")
            nc.sync.dma_start(pt_sb[:], pt[:])

            pk_sb = [ap.tile([128, PL], BF16, tag=f"pk{h}", name=f"pk{h}")
                     for h in range(HLOC)]
            pv_sb = ap.tile([PL, DLOC], BF16, tag="pv")

            agin = {}
            agout = {}
            for b in range(B):
                for h in range(HLOC):
                    agin[b, h] = dp.tile([NQT, 128, 128], BF16,
                                         tag=f"agin{b}_{h}", name=f"agin{b}_{h}")
                    agout[b, h] = dp.tile([NC, NQT, 128, 128], BF16,
                                          tag=f"agout{b}_{h}",
                                          name=f"agout{b}_{h}",
                                          addr_space="Shared")

            XT_BUFS = 4     # [128, 4096] quarters (one chunk live)
            WQK_BUFS = 2
            QK_BUFS = 6   # near-full cross-batch overlap
            V_BUFS = NQT + 2
            AG_BUFS = 2

            qT = {}
            kT = {}
            v_sb = {}

            def gen_qkv(b):
                for tc2 in range(2):
                    tcg = b * 2 + tc2
                    cols = slice(tc2 * 512, (tc2 + 1) * 512)
                    xts = []
                    for q in range(4):
                        t = sp.tile([128, 8 * 512], BF16, tag="xt", bufs=XT_BUFS,
                                    name=f"xt{tcg}_{q}")
                        nc.sync.dma_start(t[:], xt[tcg, :, 8 * q:8 * (q + 1), :])
                        xts.append(t)

                    def x_sl(i):
                        return xts[i // 8][:, (i % 8) * 512:(i % 8 + 1) * 512]

                    if tc2 == 0:
                        qT[b] = [sp.tile([128, S], BF16, tag="qT", bufs=QK_BUFS,
                                         name=f"qT{b}_{j}") for j in range(HLOC)]
                        kT[b] = [sp.tile([128, S], BF16, tag="kT", bufs=QK_BUFS,
                                         name=f"kT{b}_{j}") for j in range(HLOC)]
                        v_sb[b] = [sp.tile([128, DLOC], BF16, tag="v", bufs=V_BUFS,
                                           name=f"v{b}_{j}") for j in range(NQT)]
                    # --- q & k projections (T-major out) + RoPE ---
                    for proj, wdram, dstT in ((0, wqt, qT[b]), (1, wkt, kT[b])):
                        for dqb in range(HLOC):
                            wt = sp.tile([128, NDX * 128], BF16, tag="wqk",
                                         bufs=WQK_BUFS)
                            nc.sync.dma_start(wt[:], wdram[dqb])
                            ps = pp.tile([128, 512], F32, tag="mm512", bufs=2)
                            for i in range(NDX):
                                nc.tensor.matmul(
                                    ps[:], wt[:, i * 128:(i + 1) * 128], x_sl(i),
                                    start=(i == 0), stop=(i == NDX - 1))
                            if proj == 1 and b == 0 and tc2 == 0:
                                # prompt keys for this head, reusing wk tiles
                                psk = pp.tile([128, 512], F32, tag="sc", bufs=2)
                                for i in range(NDX):
                                    nc.tensor.matmul(
                                        psk[:, 0:PL], wt[:, i * 128:(i + 1) * 128],
                                        pt_sb[:, i * PL:(i + 1) * PL],
                                        start=(i == 0), stop=(i == NDX - 1))
                                nc.vector.tensor_copy(pk_sb[dqb][:], psk[:, 0:PL])
                            # RoPE: rows 0:64 = even hd dims, 64:128 = odd
                            c_sl = cos_sb[:, cols]
                            s_sl = sin_sb[:, cols]
                            t_rc = sp.tile([64, 512], F32, tag="rt", bufs=4)
                            t_rs = sp.tile([64, 512], F32, tag="rt", bufs=4)
                            t_ic = sp.tile([64, 512], F32, tag="rt", bufs=4)
                            t_is = sp.tile([64, 512], F32, tag="rt", bufs=4)
                            nc.vector.tensor_tensor(t_rc[:], ps[0:64, :], c_sl, op=OP.mult)
                            nc.vector.tensor_tensor(t_rs[:], ps[0:64, :], s_sl, op=OP.mult)
                            nc.vector.tensor_tensor(t_ic[:], ps[64:128, :], c_sl, op=OP.mult)
                            nc.vector.tensor_tensor(t_is[:], ps[64:128, :], s_sl, op=OP.mult)
                            nc.vector.tensor_tensor(dstT[dqb][0:64, cols], t_rc[:],
                                                    t_is[:], op=OP.subtract)
                            nc.gpsimd.tensor_tensor(dstT[dqb][64:128, cols], t_rs[:],
                                                    t_ic[:], op=OP.add)
                            yield
                    # --- v projection (natural [t, dv]) ---
                    for tblk in range(4):
                        ps = pp.tile([128, 512], F32, tag="mm512", bufs=2)
                        for i in range(NDX):
                            nc.tensor.matmul(
                                ps[:], x_sl(i)[:, tblk * 128:(tblk + 1) * 128],
                                wv_sl(i), start=(i == 0), stop=(i == NDX - 1))
                        nc.vector.tensor_copy(v_sb[b][tc2 * 4 + tblk][:], ps[:])
                        yield
                    if b == 0 and tc2 == 0:
                        psv = pp.tile([128, 512], F32, tag="mm512", bufs=2)
                        for i in range(NDX):
                            nc.tensor.matmul(psv[0:PL, :],
                                             pt_sb[:, i * PL:(i + 1) * PL],
                                             wv_sl(i),
                                             start=(i == 0), stop=(i == NDX - 1))
                        nc.vector.tensor_copy(pv_sb[:], psv[0:PL, :])

            def gen_att(b):
                for h in range(HLOC):
                    for qg in range(NQG):
                        stage = sp.tile([128, 512], BF16, tag="stage", bufs=2,
                                        name=f"stage{b}_{h}_{qg}")
                        qbase = qg * 512
                        entries = plan2[qg]
                        probs = []
                        for kb, q0, q1, adds in entries:
                            coff = q0 * 128
                            ncols = (q1 - q0 + 1) * 128
                            ssc = pp.tile([128, 512], F32, tag="sc", bufs=2)
                            nc.tensor.matmul(
                                ssc[:, coff:coff + ncols],
                                kT[b][h][:, kb * 128:(kb + 1) * 128],
                                qT[b][h][:, qbase + coff:qbase + coff + ncols],
                                start=True, stop=True)
                            for q, idx in adds:
                                nc.vector.tensor_tensor(
                                    ssc[:, q * 128:(q + 1) * 128],
                                    ssc[:, q * 128:(q + 1) * 128],
                                    mt_sb[idx][:], op=OP.add)
                            pr = sp.tile([128, 512], BF16, tag="probs", bufs=8)
                            nc.scalar.activation(pr[:, coff:coff + ncols],
                                                 ssc[:, coff:coff + ncols],
                                                 AF.Exp, scale=SCALE)
                            probs.append((kb, coff, ncols, pr))
                        # prompt scores
                        psc = pp.tile([128, 512], F32, tag="sc", bufs=2)
                        nc.tensor.matmul(psc[0:PL, :], pk_sb[h][:],
                                         qT[b][h][:, qbase:qbase + 512],
                                         start=True, stop=True)
                        ppr = sp.tile([PL, 512], BF16, tag="pprobs", bufs=1)
                        nc.scalar.activation(ppr[:], psc[0:PL, :], AF.Exp,
                                             scale=SCALE)
                        # PV accumulation + sums
                        po = pp.tile([128, 512], F32, tag="pv", bufs=2)
                        pss = pp.tile([128, 512], F32, tag="aux", bufs=2)
                        n = len(probs)
                        for i, (kb, coff, ncols, pr) in enumerate(probs):
                            nc.tensor.matmul(
                                po[:, coff:coff + ncols],
                                v_sb[b][kb][:, h * 128:(h + 1) * 128],
                                pr[:, coff:coff + ncols],
                                start=(i == 0), stop=(i == n - 1))
                        for i, (kb, coff, ncols, pr) in enumerate(probs):
                            nc.tensor.matmul(
                                pss[0:1, coff:coff + ncols], ones_col[:, 0:1],
                                pr[:, coff:coff + ncols],
                                start=(i == 0), stop=(i == n - 1))
                        ppo = pp.tile([128, 512], F32, tag="pv", bufs=2)
                        nc.tensor.matmul(ppo[:], pv_sb[0:PL, h * 128:(h + 1) * 128],
                                         ppr[:], start=True, stop=True)
                        nc.tensor.matmul(pss[32:33, :], ones_col[0:PL, 0:1], ppr[:],
                                         start=True, stop=True)
                        # reciprocals (+ gate on the prompt one)
                        recs = sp.tile([1, 1024], F32, tag="recs", bufs=1)
                        with nc.allow_low_precision("softmax denom in bf16"):
                            nc.vector.reciprocal(recs[0:1, 0:512], pss[0:1, :])
                            nc.vector.reciprocal(recs[0:1, 512:1024], pss[32:33, :])
                        nc.vector.tensor_scalar(recs[0:1, 512:1024],
                                                recs[0:1, 512:1024],
                                                gates_sb[0:1, h:h + 1], None,
                                                op0=OP.mult)
                        # broadcast row-vector across partitions (GpSimd)
                        bcs = sp.tile([128, 1024], F32, tag="bcs", bufs=1)
                        nc.gpsimd.partition_broadcast(bcs[:], recs[0:1, :])
                        po_c = sp.tile([128, 512], F32, tag="poc", bufs=2)
                        ppo_c = sp.tile([128, 512], F32, tag="poc", bufs=2)
                        nc.vector.tensor_copy(po_c[:], po[:])
                        nc.vector.tensor_copy(ppo_c[:], ppo[:])
                        t1 = sp.tile([128, 512], BF16, tag="cmb", bufs=2)
                        t2 = sp.tile([128, 512], BF16, tag="cmb", bufs=2)
                        nc.gpsimd.tensor_tensor(t1[:], po_c[:], bcs[:, 0:512],
                                                op=OP.mult)
                        nc.gpsimd.tensor_tensor(t2[:], ppo_c[:], bcs[:, 512:1024],
                                                op=OP.mult)
                        nc.vector.tensor_tensor(stage[:], t1[:], t2[:], op=OP.add)
                        nc.sync.dma_start(
                            agin[b, h][qg * 4:(qg + 1) * 4].rearrange(
                                "n p c -> p n c"), stage[:])
                        if qg == NQG - 1:
                            nc.gpsimd.collective_compute(
                                "AllGather", OP.bypass,
                                replica_groups=[list(range(NC))],
                                ins=[agin[b, h].opt()], outs=[agout[b, h].opt()])
                        yield

            def gen_oproj(b):
                for tq in range(NQT):
                    agt = sp.tile([128, NDX * 128], BF16, tag="ag",
                                  bufs=AG_BUFS, name=f"ag{b}_{tq}")
                    agt_v = agt[:].rearrange("p (n h c) -> p n h c",
                                             n=NC, h=HLOC, c=128)
                    for hl in range(HLOC):
                        nc.sync.dma_start(
                            agt_v[:, :, hl, :],
                            agout[b, hl][:, tq].rearrange("n p c -> p n c"))
                    pso = pp.tile([128, 512], F32, tag="mm512", bufs=2)
                    for i in range(NDX):
                        nc.tensor.matmul(
                            pso[:], agt[:, i * 128:(i + 1) * 128],
                            wo_sl(i), start=(i == 0), stop=(i == NDX - 1))
                    ost = sp.tile([128, 512], F32, tag="ost", bufs=1)
                    nc.vector.tensor_copy(ost[:], pso[:])
                    r0 = b * S + tq * 128
                    nc.sync.dma_start(out_d[r0:r0 + 128, :], ost[:])
                    yield

            # ---- software-pipelined emission ----
            g_qkv0 = gen_qkv(0)
            next(g_qkv0)
            emit_wvwo_loads()
            for _ in g_qkv0:
                pass
            g_att0, g_qkv1 = gen_att(0), gen_qkv(1)
            for _ in g_att0:
                for _ in range(3):
                    next(g_qkv1, None)
            for _ in g_qkv1:
                pass
            g_att1, g_o0 = gen_att(1), gen_oproj(0)
            cnt = 0
            for _ in g_att1:
                cnt += 1
                if cnt >= 4:
                    next(g_o0, None)
            for _ in g_o0:
                pass
            for _ in gen_oproj(1):
                pass

    nc.compile()
    return nc


def kernel(**inputs):
    x = np.asarray(inputs["x"], np.float32)
    wq = np.asarray(inputs["wq"], np.float32)
    wk = np.asarray(inputs["wk"], np.float32)
    wv = np.asarray(inputs["wv"], np.float32)
    wo = np.asarray(inputs["wo"], np.float32)
    prompt = np.asarray(inputs["prompt"], np.float32)
    prompt_gate = np.asarray(inputs["prompt_gate"], np.float32)
    freqs_cos = np.asarray(inputs["freqs_cos"], np.float32)
    freqs_sin = np.asarray(inputs["freqs_sin"], np.float32)
    mask = np.asarray(inputs["mask"], np.float32)

    plan, mlist = _analyze_mask(mask)
    plan2 = _group_plan(plan, len(mlist))
    n_mt = len(mlist) + 1  # + trailing -inf tile
    plan_key = (tuple(plan2), n_mt)
    if plan_key not in _PROG_CACHE:
        _PROG_CACHE[plan_key] = _build_program(plan2, n_mt)
    nc = _PROG_CACHE[plan_key]

    # ---- shared host prep ----
    perm = np.concatenate([np.arange(0, HD, 2), np.arange(1, HD, 2)])
    xT = np.ascontiguousarray(x.reshape(T, D).T.astype(NPBF16))
    # [4, 128, NDX, 512]: [tcg, dx_in_block, dx_block, t_in_chunk]
    xt_tiles = np.ascontiguousarray(
        xT.reshape(NDX, 128, 4, 512).transpose(2, 1, 0, 3))
    ptT = np.ascontiguousarray(prompt.T.astype(NPBF16))       # [D, PL]
    pt_tiles = np.ascontiguousarray(
        ptT.reshape(NDX, 128, PL).transpose(1, 0, 2))
    cosT = np.ascontiguousarray(freqs_cos.T.astype(np.float32))
    sinT = np.ascontiguousarray(freqs_sin.T.astype(np.float32))
    neg = np.full((1, 128, 128), -1e30, np.float32)
    if mlist:
        mtiles = np.concatenate([np.stack(mlist), neg])
    else:
        mtiles = neg

    def shard_qk(w, c):
        rows = np.concatenate(
            [c * DLOC + j * HD + perm for j in range(HLOC)])
        wT = w[rows, :].T.astype(NPBF16)                      # [D, DLOC]
        return np.ascontiguousarray(
            wT.reshape(NDX, 128, HLOC, 128).transpose(2, 1, 0, 3))

    def shard_rhs(w, c):
        # rows c*DLOC..+DLOC of w, transposed -> [D, DLOC] -> [128,NDX,DLOC]
        wT = w[c * DLOC:(c + 1) * DLOC, :].T.astype(NPBF16)
        return np.ascontiguousarray(wT.reshape(NDX, 128, DLOC).transpose(1, 0, 2))

    in_maps = []
    for c in range(NC):
        in_maps.append(dict(
            xt=xt_tiles,
            wqt=shard_qk(wq, c),
            wkt=shard_qk(wk, c),
            wvt=shard_rhs(wv, c),
            wot=shard_rhs(wo, c),
            pt=pt_tiles,
            cosT=cosT,
            sinT=sinT,
            gates=np.ascontiguousarray(
                prompt_gate.reshape(H)[c * HLOC:(c + 1) * HLOC][None, :]
            ).astype(np.float32),
            mtiles=mtiles,
        ))

    res = bass_utils.run_bass_kernel_spmd(
        nc, in_maps, core_ids=list(range(NC)),
        trace=bool(os.environ.get("BASS_TRACE")))
    kernel.last_result = res

    full = np.empty((T, D), np.float32)
    for c in range(NC):
        full[:, c * DLOC:(c + 1) * DLOC] = res.results[c]["out"]
    return full.reshape(B, S, D)


# revision 11
# speedup vs baseline: 1.0976x; 1.0976x over previous
"""Trainium2 Bass kernel: multi-head attention with RoPE + gated prompt
injection (nn_Attention_28080496181816), sharded over 8 NeuronCores.

Sharding: tensor-parallel over heads. Core c owns heads [4c, 4c+4):
  - wq/wk/wv column-sharded (per-head), o-proj via AllGather of the
    per-core attention outputs + column-sharded wo matmul.
  - Host-side unshard is a pure concatenation of output column slices.

Layout: "T-major" — activations live as [feature, token] on device so
every matmul contraction lands on the partition axis with no on-device
transposes. RoPE pairs are made contiguous by permuting wq/wk rows
(per head: even hd dims then odd hd dims) on the host.

Attention runs in 512-wide query groups with variable-width score
matmuls derived from the runtime mask structure (causal -> exact
lower-triangle work). The emission order software-pipelines phases so
the in-order PE stream interleaves attention with the next batch's
projections and the previous batch's output projection.
"""

import math
import os
import sys
import types

import numpy as np
import ml_dtypes

# --- optional NTFF profile hook shim (only needed if BASS_TRACE is set;
# the stock image lacks antenv.axon_hooks) ---
try:
    import antenv.axon_hooks  # noqa: F401
except Exception:
    try:
        import antenv
        _m = types.ModuleType("antenv.axon_hooks")
        _hook = [None]
        _m.set_axon_ntff_profile_hook = lambda h: _hook.__setitem__(0, h)
        _m.get_axon_ntff_profile_hook = lambda: _hook[0]
        sys.modules["antenv.axon_hooks"] = _m
        antenv.axon_hooks = _m
        from trn_agent_boot.trn_boot import _ntff_profile_via_ctypes
        _p = _ntff_profile_via_ctypes("/opt/axon/libaxon_pjrt.so")
        if _p is not None:
            _m.set_axon_ntff_profile_hook(_p)
    except Exception:
        pass

import concourse.bacc as bacc
import concourse.mybir as mybir
import concourse.tile as tile
from concourse import bass_utils

BF16 = mybir.dt.bfloat16
F32 = mybir.dt.float32
NPBF16 = ml_dtypes.bfloat16

B, S, D, H, HD, PL = 2, 1024, 4096, 32, 128, 10
NC = 8              # cores
HLOC = H // NC      # 4 heads per core
DLOC = HLOC * HD    # 512
T = B * S           # 2048
NDX = D // 128      # 32 contraction blocks
NQT = S // 128      # 8 query tiles per batch
NQG = NQT // 4      # 2 query groups of 512
SCALE = 1.0 / math.sqrt(HD)

_PROG_CACHE = {}


def _analyze_mask(mask):
    """Classify each 128x128 tile of the additive mask: skip (fully
    masked), clear (all zero) or mixed (ship the transposed, pre-scaled
    tile). Deduplicates mixed tiles."""
    mq = np.asarray(mask).reshape(S, S)
    plan = []
    uniq = {}
    mlist = []
    for qi in range(NQT):
        row = []
        for kb in range(NQT):
            sub = mq[qi * 128:(qi + 1) * 128, kb * 128:(kb + 1) * 128]
            if np.all(sub <= -1e8):
                continue
            if np.all(sub == 0):
                row.append((kb, None))
                continue
            tt = np.ascontiguousarray(sub.T.astype(np.float32) / SCALE)
            key = tt.tobytes()
            if key not in uniq:
                uniq[key] = len(mlist)
                mlist.append(tt)
            row.append((kb, uniq[key]))
        plan.append(row)
    return plan, mlist


def _group_plan(plan, n_mtiles):
    """512-wide query groups. Per group: list of (kb, q0, q1, adds) with
    q0..q1 the covered query quarters and adds = [(quarter, mtile_idx)];
    mtile_idx == n_mtiles selects the -inf tile. The first kb of each
    group always spans the full group so PSUM has_written is set."""
    NEG = n_mtiles
    plan2 = []
    for qg in range(NQG):
        qmode = []
        for q in range(4):
            qmode.append(dict(plan[qg * 4 + q]))
        live = sorted(set().union(*[set(d.keys()) for d in qmode]))
        entries = []
        for j, kb in enumerate(live):
            pres = [kb in qmode[q] for q in range(4)]
            if j == 0:
                q0, q1 = 0, 3
            else:
                q0 = min(q for q in range(4) if pres[q])
                q1 = max(q for q in range(4) if pres[q])
            adds = []
            for q in range(q0, q1 + 1):
                if not pres[q]:
                    adds.append((q, NEG))
                elif qmode[q][kb] is not None:
                    adds.append((q, qmode[q][kb]))
            entries.append((kb, q0, q1, tuple(adds)))
        plan2.append(tuple(entries))
    return plan2


def _build_program(plan2, n_mt):
    """Build + compile the SPMD program (identical on all 8 cores).
    n_mt counts mask tiles INCLUDING the trailing -inf tile."""
    nc = bacc.Bacc("TRN2", target_bir_lowering=False, debug=False, num_devices=NC)

    # p-major host layouts so each logical group is ONE big DMA
    xt = nc.dram_tensor("xt", [4, 128, NDX, 512], BF16, kind="ExternalInput")
    wqt = nc.dram_tensor("wqt", [HLOC, 128, NDX, 128], BF16, kind="ExternalInput")
    wkt = nc.dram_tensor("wkt", [HLOC, 128, NDX, 128], BF16, kind="ExternalInput")
    wvt = nc.dram_tensor("wvt", [128, NDX, DLOC], BF16, kind="ExternalInput")
    wot = nc.dram_tensor("wot", [128, NDX, DLOC], BF16, kind="ExternalInput")
    pt = nc.dram_tensor("pt", [128, NDX, PL], BF16, kind="ExternalInput")
    cosT = nc.dram_tensor("cosT", [64, S], F32, kind="ExternalInput")
    sinT = nc.dram_tensor("sinT", [64, S], F32, kind="ExternalInput")
    gates = nc.dram_tensor("gates", [PL, HLOC], F32, kind="ExternalInput")
    mtiles = nc.dram_tensor("mtiles", [n_mt, 128, 128], F32, kind="ExternalInput")
    out_d = nc.dram_tensor("out", [T, DLOC], F32, kind="ExternalOutput")

    AF = mybir.ActivationFunctionType
    OP = mybir.AluOpType

    with tile.TileContext(nc) as tc:
        with (
            tc.tile_pool(name="const", bufs=1) as cpool,
            tc.tile_pool(name="wres", bufs=1) as wres,
            tc.tile_pool(name="stream", bufs=1) as sp,
            tc.tile_pool(name="act", bufs=1) as ap,
            tc.tile_pool(name="psum", bufs=1, space="PSUM") as pp,
            tc.tile_pool(name="dram", bufs=1, space="DRAM") as dp,
        ):
            # ---- persistent constants / weights ----
            cos_sb = cpool.tile([64, S], F32, tag="cos")
            sin_sb = cpool.tile([64, S], F32, tag="sin")
            nc.sync.dma_start(cos_sb[:], cosT[:])
            nc.sync.dma_start(sin_sb[:], sinT[:])
            gates_sb = cpool.tile([PL, HLOC], F32, tag="gates")
            nc.sync.dma_start(gates_sb[:], gates[:])
            mt_sb = []
            for i in range(n_mt):
                t = cpool.tile([128, 128], F32, tag=f"mt{i}", name=f"mt{i}")
                nc.sync.dma_start(t[:], mtiles[i])
                mt_sb.append(t)
            ones_col = cpool.tile([128, 1], BF16, tag="ones_col")
            nc.vector.memset(ones_col[:], 1.0)

            # wv / wo resident: 4 tiles each of [128, 8*512]
            # (DMAs issued on the GpSimd queue after the first projection
            # group so they don't delay the critical-path x/wq loads)
            wv_sb = [wres.tile([128, 8 * DLOC], BF16, tag=f"wv{j}",
                               name=f"wv{j}") for j in range(4)]
            wo_sb = [wres.tile([128, 8 * DLOC], BF16, tag=f"wo{j}",
                               name=f"wo{j}") for j in range(4)]

            def emit_wvwo_loads():
                for j in range(4):
                    nc.gpsimd.dma_start(wv_sb[j][:], wvt[:, 8 * j:8 * (j + 1), :])
                for j in range(4):
                    nc.gpsimd.dma_start(wo_sb[j][:], wot[:, 8 * j:8 * (j + 1), :])

            def wv_sl(i):
                return wv_sb[i // 8][:, (i % 8) * DLOC:(i % 8 + 1) * DLOC]

            def wo_sl(i):
                return wo_sb[i // 8][:, (i % 8) * DLOC:(i % 8 + 1) * DLOC]

            pt_sb = cpool.tile([128, NDX * PL], BF16, tag="pt")
            nc.sync.dma_start(pt_sb[:], pt[:])

            pk_sb = [ap.tile([128, PL], BF16, tag=f"pk{h}", name=f"pk{h}")
                     for h in range(HLOC)]
            pv_sb = ap.tile([PL, DLOC], BF16, tag="pv")

            agin = {}
            agout = {}
            for b in range(B):
                for h in range(HLOC):
                    agin[b, h] = dp.tile([NQT, 128, 128], BF16,
                                         tag=f"agin{b}_{h}", name=f"agin{b}_{h}")
                    agout[b, h] = dp.tile([NC, NQT, 128, 128], BF16,
                                          tag=f"agout{b}_{h}",
                                          name=f"agout{b}_{h}",
                                          addr_space="Shared")

            XT_BUFS = 4     # [128, 4096] quarters (one chunk live)
            WQK_BUFS = 2
            QK_BUFS = 6   # near-full cross-batch overlap
            V_BUFS = NQT + 4
            AG_BUFS = 2

            qT = {}
            kT = {}
            v_sb = {}

            def gen_qkv(b):
                for tc2 in range(2):
                    tcg = b * 2 + tc2
                    cols = slice(tc2 * 512, (tc2 + 1) * 512)
                    xts = [sp.tile([128, 8 * 512], BF16, tag="xt",
                                   bufs=XT_BUFS, name=f"xt{tcg}_{q}")
                           for q in range(4)]
                    nc.sync.dma_start(xts[0][:], xt[tcg, :, 0:8, :])
                    xlate = [(q, xts[q]) for q in range(1, 4)]

                    def x_sl(i):
                        return xts[i // 8][:, (i % 8) * 512:(i % 8 + 1) * 512]

                    if tc2 == 0:
                        qT[b] = [sp.tile([128, S], BF16, tag="qT", bufs=QK_BUFS,
                                         name=f"qT{b}_{j}") for j in range(HLOC)]
                        kT[b] = [sp.tile([128, S], BF16, tag="kT", bufs=QK_BUFS,
                                         name=f"kT{b}_{j}") for j in range(HLOC)]
                        v_sb[b] = [sp.tile([128, DLOC], BF16, tag="v", bufs=V_BUFS,
                                           name=f"v{b}_{j}") for j in range(NQT)]
                    # --- q & k projections (T-major out) + RoPE ---
                    for proj, wdram, dstT in ((0, wqt, qT[b]), (1, wkt, kT[b])):
                        for dqb in range(HLOC):
                            wt = sp.tile([128, NDX * 128], BF16, tag="wqk",
                                         bufs=WQK_BUFS)
                            nc.sync.dma_start(wt[:], wdram[dqb])
                            while xlate:
                                q, xtile = xlate.pop(0)
                                nc.sync.dma_start(
                                    xtile[:], xt[tcg, :, 8 * q:8 * (q + 1), :])
                            ps = pp.tile([128, 512], F32, tag="mm512", bufs=2)
                            for i in range(NDX):
                                nc.tensor.matmul(
                                    ps[:], wt[:, i * 128:(i + 1) * 128], x_sl(i),
                                    start=(i == 0), stop=(i == NDX - 1))
                            if proj == 1 and b == 0 and tc2 == 0:
                                # prompt keys for this head, reusing wk tiles
                                psk = pp.tile([128, 512], F32, tag="sc", bufs=2)
                                for i in range(NDX):
                                    nc.tensor.matmul(
                                        psk[:, 0:PL], wt[:, i * 128:(i + 1) * 128],
                                        pt_sb[:, i * PL:(i + 1) * PL],
                                        start=(i == 0), stop=(i == NDX - 1))
                                nc.vector.tensor_copy(pk_sb[dqb][:], psk[:, 0:PL])
                            # RoPE: rows 0:64 = even hd dims, 64:128 = odd
                            c_sl = cos_sb[:, cols]
                            s_sl = sin_sb[:, cols]
                            t_rc = sp.tile([64, 512], F32, tag="rt", bufs=4)
                            t_rs = sp.tile([64, 512], F32, tag="rt", bufs=4)
                            t_ic = sp.tile([64, 512], F32, tag="rt", bufs=4)
                            t_is = sp.tile([64, 512], F32, tag="rt", bufs=4)
                            nc.vector.tensor_tensor(t_rc[:], ps[0:64, :], c_sl, op=OP.mult)
                            nc.vector.tensor_tensor(t_rs[:], ps[0:64, :], s_sl, op=OP.mult)
                            nc.vector.tensor_tensor(t_ic[:], ps[64:128, :], c_sl, op=OP.mult)
                            nc.vector.tensor_tensor(t_is[:], ps[64:128, :], s_sl, op=OP.mult)
                            nc.vector.tensor_tensor(dstT[dqb][0:64, cols], t_rc[:],
                                                    t_is[:], op=OP.subtract)
                            nc.gpsimd.tensor_tensor(dstT[dqb][64:128, cols], t_rs[:],
                                                    t_ic[:], op=OP.add)
                            yield
                    # --- v projection (natural [t, dv]) ---
                    for tblk in range(4):
                        ps = pp.tile([128, 512], F32, tag="mm512", bufs=2)
                        for i in range(NDX):
                            nc.tensor.matmul(
                                ps[:], x_sl(i)[:, tblk * 128:(tblk + 1) * 128],
                                wv_sl(i), start=(i == 0), stop=(i == NDX - 1))
                        nc.vector.tensor_copy(v_sb[b][tc2 * 4 + tblk][:], ps[:])
                        yield
                    if b == 0 and tc2 == 0:
                        psv = pp.tile([128, 512], F32, tag="mm512", bufs=2)
                        for i in range(NDX):
                            nc.tensor.matmul(psv[0:PL, :],
                                             pt_sb[:, i * PL:(i + 1) * PL],
                                             wv_sl(i),
                                             start=(i == 0), stop=(i == NDX - 1))
                        nc.vector.tensor_copy(pv_sb[:], psv[0:PL, :])
                        for hh in range(HLOC):
                            nc.vector.tensor_scalar(
                                pv_sb[0:PL, hh * 128:(hh + 1) * 128],
                                pv_sb[0:PL, hh * 128:(hh + 1) * 128],
                                gates_sb[0:PL, hh:hh + 1], None, op0=OP.mult)

            def gen_att(b):
                for h in range(HLOC):
                    for qg in range(NQG):
                        stage = sp.tile([128, 512], BF16, tag="stage", bufs=2,
                                        name=f"stage{b}_{h}_{qg}")
                        qbase = qg * 512
                        entries = plan2[qg]
                        probs = []
                        for kb, q0, q1, adds in entries:
                            coff = q0 * 128
                            ncols = (q1 - q0 + 1) * 128
                            ssc = pp.tile([128, 512], F32, tag="sc", bufs=2)
                            nc.tensor.matmul(
                                ssc[:, coff:coff + ncols],
                                kT[b][h][:, kb * 128:(kb + 1) * 128],
                                qT[b][h][:, qbase + coff:qbase + coff + ncols],
                                start=True, stop=True)
                            for q, idx in adds:
                                nc.vector.tensor_tensor(
                                    ssc[:, q * 128:(q + 1) * 128],
                                    ssc[:, q * 128:(q + 1) * 128],
                                    mt_sb[idx][:], op=OP.add)
                            pr = sp.tile([128, 512], BF16, tag="probs", bufs=8)
                            nc.scalar.activation(pr[:, coff:coff + ncols],
                                                 ssc[:, coff:coff + ncols],
                                                 AF.Exp, scale=SCALE)
                            probs.append((kb, coff, ncols, pr))
                        # prompt scores
                        psc = pp.tile([128, 512], F32, tag="sc", bufs=2)
                        nc.tensor.matmul(psc[0:PL, :], pk_sb[h][:],
                                         qT[b][h][:, qbase:qbase + 512],
                                         start=True, stop=True)
                        ppr = sp.tile([PL, 512], BF16, tag="pprobs", bufs=1)
                        nc.scalar.activation(ppr[:], psc[0:PL, :], AF.Exp,
                                             scale=SCALE)
                        # PV accumulation + sums
                        po = pp.tile([128, 512], F32, tag="pv", bufs=2)
                        pss = pp.tile([128, 512], F32, tag="aux", bufs=2)
                        n = len(probs)
                        for i, (kb, coff, ncols, pr) in enumerate(probs):
                            nc.tensor.matmul(
                                po[:, coff:coff + ncols],
                                v_sb[b][kb][:, h * 128:(h + 1) * 128],
                                pr[:, coff:coff + ncols],
                                start=(i == 0), stop=(i == n - 1))
                        for i, (kb, coff, ncols, pr) in enumerate(probs):
                            nc.tensor.matmul(
                                pss[0:1, coff:coff + ncols], ones_col[:, 0:1],
                                pr[:, coff:coff + ncols],
                                start=(i == 0), stop=(i == n - 1))
                        ppo = pp.tile([128, 512], F32, tag="pv", bufs=2)
                        nc.tensor.matmul(ppo[:], pv_sb[0:PL, h * 128:(h + 1) * 128],
                                         ppr[:], start=True, stop=True)
                        nc.tensor.matmul(pss[32:33, :], ones_col[0:PL, 0:1], ppr[:],
                                         start=True, stop=True)
                        # reciprocals (+ gate on the prompt one)
                        recs = sp.tile([1, 1024], BF16, tag="recs", bufs=2)
                        with nc.allow_low_precision("softmax denom in bf16"):
                            nc.vector.reciprocal(recs[0:1, 0:512], pss[0:1, :])
                            nc.vector.reciprocal(recs[0:1, 512:1024], pss[32:33, :])
                        # broadcast row-vector across partitions (GpSimd)
                        bcs = sp.tile([128, 1024], BF16, tag="bcs", bufs=2)
                        nc.gpsimd.partition_broadcast(bcs[:], recs[0:1, :])
                        po_c = sp.tile([128, 512], BF16, tag="poc", bufs=2)
                        ppo_c = sp.tile([128, 512], BF16, tag="poc", bufs=2)
                        nc.vector.tensor_copy(po_c[:], po[:])
                        nc.vector.tensor_copy(ppo_c[:], ppo[:])
                        t1 = sp.tile([128, 512], BF16, tag="cmb", bufs=2)
                        t2 = sp.tile([128, 512], BF16, tag="cmb", bufs=2)
                        nc.gpsimd.tensor_tensor(t1[:], po_c[:], bcs[:, 0:512],
                                                op=OP.mult)
                        nc.gpsimd.tensor_tensor(t2[:], ppo_c[:], bcs[:, 512:1024],
                                                op=OP.mult)
                        nc.vector.tensor_tensor(stage[:], t1[:], t2[:], op=OP.add)
                        nc.sync.dma_start(
                            agin[b, h][qg * 4:(qg + 1) * 4].rearrange(
                                "n p c -> p n c"), stage[:])
                        if qg == NQG - 1:
                            nc.gpsimd.collective_compute(
                                "AllGather", OP.bypass,
                                replica_groups=[list(range(NC))],
                                ins=[agin[b, h].opt()], outs=[agout[b, h].opt()])
                        yield

            def gen_oproj(b):
                for tq in range(NQT):
                    agt = sp.tile([128, NDX * 128], BF16, tag="ag",
                                  bufs=AG_BUFS, name=f"ag{b}_{tq}")
                    agt_v = agt[:].rearrange("p (n h c) -> p n h c",
                                             n=NC, h=HLOC, c=128)
                    for hl in range(HLOC):
                        nc.sync.dma_start(
                            agt_v[:, :, hl, :],
                            agout[b, hl][:, tq].rearrange("n p c -> p n c"))
                    pso = pp.tile([128, 512], F32, tag="mm512", bufs=2)
                    for i in range(NDX):
                        nc.tensor.matmul(
                            pso[:], agt[:, i * 128:(i + 1) * 128],
                            wo_sl(i), start=(i == 0), stop=(i == NDX - 1))
                    ost = sp.tile([128, 512], F32, tag="ost", bufs=1)
                    nc.vector.tensor_copy(ost[:], pso[:])
                    r0 = b * S + tq * 128
                    nc.sync.dma_start(out_d[r0:r0 + 128, :], ost[:])
                    yield

            # ---- software-pipelined emission ----
            g_qkv0 = gen_qkv(0)
            next(g_qkv0)
            emit_wvwo_loads()
            for _ in g_qkv0:
                pass
            g_att0, g_qkv1 = gen_att(0), gen_qkv(1)
            for _ in g_att0:
                for _ in range(3):
                    next(g_qkv1, None)
            for _ in g_qkv1:
                pass
            g_att1, g_o0 = gen_att(1), gen_oproj(0)
            for _ in g_att1:
                next(g_o0, None)
            for _ in g_o0:
                pass
            for _ in gen_oproj(1):
                pass

    nc.compile()
    return nc


def kernel(**inputs):
    x = np.asarray(inputs["x"], np.float32)
    wq = np.asarray(inputs["wq"], np.float32)
    wk = np.asarray(inputs["wk"], np.float32)
    wv = np.asarray(inputs["wv"], np.float32)
    wo = np.asarray(inputs["wo"], np.float32)
    prompt = np.asarray(inputs["prompt"], np.float32)
    prompt_gate = np.asarray(inputs["prompt_gate"], np.float32)
    freqs_cos = np.asarray(inputs["freqs_cos"], np.float32)
    freqs_sin = np.asarray(inputs["freqs_sin"], np.float32)
    mask = np.asarray(inputs["mask"], np.float32)

    plan, mlist = _analyze_mask(mask)
    plan2 = _group_plan(plan, len(mlist))
    n_mt = len(mlist) + 1  # + trailing -inf tile
    plan_key = (tuple(plan2), n_mt)
    if plan_key not in _PROG_CACHE:
        _PROG_CACHE[plan_key] = _build_program(plan2, n_mt)
    nc = _PROG_CACHE[plan_key]

    # ---- shared host prep ----
    perm = np.concatenate([np.arange(0, HD, 2), np.arange(1, HD, 2)])
    xT = np.ascontiguousarray(x.reshape(T, D).T.astype(NPBF16))
    # [4, 128, NDX, 512]: [tcg, dx_in_block, dx_block, t_in_chunk]
    xt_tiles = np.ascontiguousarray(
        xT.reshape(NDX, 128, 4, 512).transpose(2, 1, 0, 3))
    ptT = np.ascontiguousarray(prompt.T.astype(NPBF16))       # [D, PL]
    pt_tiles = np.ascontiguousarray(
        ptT.reshape(NDX, 128, PL).transpose(1, 0, 2))
    cosT = np.ascontiguousarray(freqs_cos.T.astype(np.float32))
    sinT = np.ascontiguousarray(freqs_sin.T.astype(np.float32))
    neg = np.full((1, 128, 128), -1e30, np.float32)
    if mlist:
        mtiles = np.concatenate([np.stack(mlist), neg])
    else:
        mtiles = neg

    def shard_qk(w, c):
        rows = np.concatenate(
            [c * DLOC + j * HD + perm for j in range(HLOC)])
        wT = w[rows, :].T.astype(NPBF16)                      # [D, DLOC]
        return np.ascontiguousarray(
            wT.reshape(NDX, 128, HLOC, 128).transpose(2, 1, 0, 3))

    def shard_rhs(w, c):
        # rows c*DLOC..+DLOC of w, transposed -> [D, DLOC] -> [128,NDX,DLOC]
        wT = w[c * DLOC:(c + 1) * DLOC, :].T.astype(NPBF16)
        return np.ascontiguousarray(wT.reshape(NDX, 128, DLOC).transpose(1, 0, 2))

    in_maps = []
    for c in range(NC):
        in_maps.append(dict(
            xt=xt_tiles,
            wqt=shard_qk(wq, c),
            wkt=shard_qk(wk, c),
            wvt=shard_rhs(wv, c),
            wot=shard_rhs(wo, c),
            pt=pt_tiles,
            cosT=cosT,
            sinT=sinT,
            gates=np.ascontiguousarray(np.repeat(
                prompt_gate.reshape(H)[c * HLOC:(c + 1) * HLOC][None, :],
                PL, axis=0)).astype(np.float32),
            mtiles=mtiles,
        ))

    res = bass_utils.run_bass_kernel_spmd(
        nc, in_maps, core_ids=list(range(NC)),
        trace=bool(os.environ.get("BASS_TRACE")))
    kernel.last_result = res

    full = np.empty((T, D), np.float32)
    for c in range(NC):
        full[:, c * DLOC:(c + 1) * DLOC] = res.results[c]["out"]
    return full.reshape(B, S, D)
